# revision 13
# baseline (speedup 1.0000x reference)
"""DifColorQuantization Trainium2 kernel (v1.6).

Math (per pixel p, codebook color k):
    ref:  argmin_k sqrt(sum_c (x_c - cb_kc + eps)^2 + eps) ; out = cb[argmin]
    sqrt/+eps are monotone, so rank by the k-dependent affine part
    s_k = sum_c w_kc x_c + b_k,  w_kc = 2(eps-cb_kc), b_k = sum_c (eps-cb_kc)^2.

Pipeline per core (H sharded 8 ways, 131072 px/core = 32768 j-columns
x 4 slots; supertile = 1024 j-cols = 4096 px; block = 128 j-cols):
    1. image resident in SBUF as bf16 2-term splits: rows
       [xh(12) xl(12) xh(12) ones(3)] x 2 column-groups at partition
       bases 0/64 -> [103, 16384] (partition-fat DMA: ~13us instead of
       50us for the old [13, L] f32 layout)
    2. scores: per block ONE bf16 matmul, lhsT = img chunk [39,128]
       stationary, rhs = folded weights [39, 128 (q,k)]:
       wh*xh + wh*xl + wl*xh + (bh+bl+bl2) ~= w*x + b to ~3e-5 abs
       -> PSUM f32 [128 px-col, (q,k)] at 1 cyc/col (4x faster than
       the baseline's fp32 matmul; measured 101 argmin flips, 2.8e-3)
    3. DVE min over k segments -> m [128, 32] f32 (full f32 compare
       precision: fp16/bf16-quantized scores measured to FAIL the 2e-2
       gate at 0.16/0.53 rel from tie multi-hots)
    4. DVE is_equal(scores, m bcast) -> one-hot bf16 SBUF
    5. PE transpose per block -> PSUM [(q,k), px]; ACT evict -> SBUF
    6. gather: 8 chunked bf16 matmuls, lhsT = cb table [128, 12], out
       partition bases rotating 0/32/64 -> colors PSUM [76, 384]
       (partition-fat: evict 463ns vs 996ns skinny); ACT evict; DMA
       out y[76, 384*32] (fat out-DMA, ~6us total).

Engine budget per supertile (cost model, max pstate): PE 1330ns
(scores 427 + transpose 427 + gather 427), DVE 2474 (min 1237 +
is_equal 1237; both PSUM-sourced f32 passes - the bound), ACT 1524
(one-hot evict 1029 + colors evict 495). gpsimd is unusable here (no
PSUM port). TimelineSim one-shot: 91.3us vs baseline 144.6us; device
For_i med-diff (quiet window): 85us/rep vs baseline 866us/rep.

Numerics: scores differ from the fp32 reference by ~3e-5 -> 99 of 1M
pixels flip to a near-equidistant color; colors pass through one bf16
rounding (0.2%). Measured end-to-end rel-l2: 3.1e-3 (< 2e-2 gate).
"""

import numpy as np

H = 1024
W = 1024
K = 32
EPS = 1e-6
NCORES = 8
ROWS = H // NCORES            # 128 rows per core
NPX = ROWS * W                # 131072 pixels per core
NSLOT = 4
SLOT_N = 512                  # columns per slot-tile
TILE_PX = NSLOT * SLOT_N      # 2048 px per packing tile
NT = NPX // TILE_PX           # 64 packing tiles
NJ = NPX // NSLOT             # 32768 j-columns
SUP = 1024                    # j-cols per supertile
NSUP = NJ // SUP              # 32 supertiles
XR = 39                       # x-fold rows per group
NGRP = 2                      # column groups (partition bases 0, 64)
GBASE = 64                    # partition base stride between groups
GCOLS = 16384                 # cols per group (16 supertiles)
GSUP = 16                     # supertiles per group


def _build_program(n_sup=NSUP, reps=1, oh_evict="act"):
    import concourse.bass as bass
    import concourse.bacc as bacc
    import concourse.tile as tile
    from concourse import mybir

    f32 = mybir.dt.float32
    bf16 = mybir.dt.bfloat16

    nc = bacc.Bacc(None, target_bir_lowering=False)
    # x-fold: 2 groups of 39 rows at partition bases 0/64 -> [103, 16384]
    x = nc.dram_tensor("x", [GBASE + XR, GCOLS], bf16, kind="ExternalInput")
    # consts bf16: [0:128] identity, [128:256] Wfold (rows 0-38 and a
    # copy at rows 64-102 to satisfy lhsT/rhs base-partition matching),
    # [256:268] gather cb table (rows = (q,k))
    cb16 = nc.dram_tensor("cb16", [128, 272], bf16, kind="ExternalInput")
    y = nc.dram_tensor("y", [76, 384 * NSUP], f32, kind="ExternalOutput")

    with tile.TileContext(nc) as tc:
        with (
            tc.tile_pool(name="const", bufs=1) as constp,
            tc.tile_pool(name="io", bufs=1) as iop,
            tc.tile_pool(name="work", bufs=4) as workp,
            tc.tile_pool(name="ps", bufs=2, space=bass.MemorySpace.PSUM) as psp,
            tc.tile_pool(name="pso", bufs=2, space=bass.MemorySpace.PSUM) as psop,
            tc.tile_pool(name="psc", bufs=2, space=bass.MemorySpace.PSUM) as pscp,
        ):
            cons_t = constp.tile([128, 272], bf16)
            nc.sync.dma_start(cons_t[:], cb16[:])
            iden_t = cons_t[:, 0:128]
            wfold_g = [
                cons_t[0:XR, 128:256],
                cons_t[GBASE : GBASE + XR, 128:256],
            ]
            gbd1_t = cons_t[:, 256:268]

            # image in 8 column-eighth tiles: compute on the first
            # supertile starts after ~1.6us of input DMA instead of
            # waiting for the whole 12.6us transfer
            QC = GCOLS // 8
            imgq = []
            for qq in range(8):
                t_ = iop.tile([GBASE + XR, QC], bf16, tag=f"img{qq}")
                nc.sync.dma_start(t_[:], x[:, QC * qq : QC * (qq + 1)])
                imgq.append(t_)

            def _super(s):
                # group and in-group column base for this supertile
                g = s // GSUP
                col0 = (s % GSUP) * SUP

                # scores: 8 blocks of [128 px-col, (q,k)], one bf16
                # matmul each (2-term folded product + 3-term bias)
                qq, qcol = col0 // QC, col0 % QC
                img = imgq[qq]
                ps_T = psp.tile([128, SUP], f32, tag="ps_T")
                for b in range(8):
                    cb_ = qcol + 128 * b
                    nc.tensor.matmul(
                        ps_T[:, 128 * b : 128 * (b + 1)],
                        img[GBASE * g : GBASE * g + XR, cb_ : cb_ + 128],
                        wfold_g[g],
                    )

                # per-pixel min over the 32 scores (f32, PSUM source)
                m = workp.tile([128, 32], f32, tag="m")
                nc.vector.tensor_reduce(
                    m[:],
                    ps_T[:].rearrange("p (s k) -> p s k", k=K),
                    axis=mybir.AxisListType.X,
                    op=mybir.AluOpType.min,
                )

                # one-hot (transposed layout): m broadcast along k
                onehot = workp.tile([128, SUP], bf16, tag="onehot")
                nc.vector.tensor_tensor(
                    onehot[:].rearrange("p (s k) -> p s k", k=K),
                    ps_T[:].rearrange("p (s k) -> p s k", k=K),
                    m[:].to_broadcast((128, 32, K)),
                    op=mybir.AluOpType.is_equal,
                )

                # transpose back to [(q,k), px] per block
                ps_O = psop.tile([128, SUP], bf16, tag="ps_O")
                for b in range(8):
                    nc.tensor.transpose(
                        ps_O[:, 128 * b : 128 * (b + 1)],
                        onehot[:, 128 * b : 128 * (b + 1)],
                        iden_t,
                    )
                oh_sb = workp.tile([128, SUP], bf16, tag="oh_sb")
                if oh_evict == "act":
                    nc.scalar.activation(
                        oh_sb[:], ps_O[:], mybir.ActivationFunctionType.Copy
                    )
                elif oh_evict == "dve":
                    nc.vector.tensor_copy(oh_sb[:], ps_O[:])
                else:  # split: ACT low half, DVE high half
                    nc.scalar.activation(
                        oh_sb[:, 0:512],
                        ps_O[:, 0:512],
                        mybir.ActivationFunctionType.Copy,
                    )
                    nc.vector.tensor_copy(oh_sb[:, 512:1024], ps_O[:, 512:1024])

                # gather colors, partition-fat-ish [76, 384]: block h ->
                # rows 32*(h%3)..+12 (PSUM matmul base must be 0/32/64),
                # cols 128*(h//3)..+128
                pc = pscp.tile([76, 384], f32, tag="pc")
                for h in range(8):
                    pb, cc = 32 * (h % 3), 128 * (h // 3)
                    nc.tensor.matmul(
                        pc[pb : pb + 12, cc : cc + 128],
                        gbd1_t,
                        oh_sb[:, 128 * h : 128 * (h + 1)],
                    )
                colors = workp.tile([76, 384], f32, tag="colors")
                nc.scalar.activation(
                    colors[:], pc[:], mybir.ActivationFunctionType.Copy
                )
                nc.sync.dma_start(y[:, 384 * s : 384 * (s + 1)], colors[:])

            def _body():
                for s in range(n_sup):
                    _super(s)

            if reps == 1:
                _body()
            else:
                with tc.For_i(0, reps, 1):
                    _body()
    nc.compile()
    return nc


def _host_consts(printability_array):
    """Pack constants: [128, 272] bf16 = identity | Wfold | gather cb."""
    import ml_dtypes

    cb = printability_array.reshape(K, 3).astype(np.float64)
    w = 2.0 * (EPS - cb)                                  # [K, 3]
    b = np.sum((EPS - cb) ** 2, axis=1)                   # [K]

    def s16(a):
        return a.astype(np.float32).astype(ml_dtypes.bfloat16)

    wh = s16(w)
    wl = s16(w - wh.astype(np.float64))
    bh = s16(b)
    bl = s16(b - bh.astype(np.float64))
    bl2 = s16(b - bh.astype(np.float64) - bl.astype(np.float64))
    cbf = cb.astype(np.float32)

    consts = np.zeros((128, 272), ml_dtypes.bfloat16)
    consts[:, 0:128] = np.eye(128, dtype=ml_dtypes.bfloat16)
    for q in range(NSLOT):
        for k in range(K):
            p = 32 * q + k
            for c in range(3):
                r = 4 * c + q
                consts[r, 128 + p] = wh[k, c]           # pairs with xh
                consts[12 + r, 128 + p] = wh[k, c]      # pairs with xl
                consts[24 + r, 128 + p] = wl[k, c]      # pairs with xh
                consts[p, 256 + r] = ml_dtypes.bfloat16(cbf[k, c])
            consts[36, 128 + p] = bh[k]
            consts[37, 128 + p] = bl[k]
            consts[38, 128 + p] = bl2[k]
    consts[64:103, 128:256] = consts[0:39, 128:256]     # base-64 copy
    return consts


def _pack_x(flat3):
    """[3, npx] f32 -> x-fold bf16 [103, 16384].

    j-column J = 512t + n holds pixels 2048t + 512q + n (q = slot).
    Rows per group: xh[4c+q] (12), xl (12), xh dup (12), ones (3).
    Group 0 (j-cols [0:16384)) at partitions 0:39, group 1 at 64:103.
    """
    import ml_dtypes

    v = flat3.reshape(3, NT, NSLOT, SLOT_N)          # (c, t, q, n)
    x12 = v.transpose(0, 2, 1, 3).reshape(12, NJ)    # row = 4c+q
    xh = x12.astype(ml_dtypes.bfloat16)
    xl = (x12 - xh.astype(np.float32)).astype(ml_dtypes.bfloat16)
    grp = np.zeros((XR, NJ), ml_dtypes.bfloat16)
    grp[0:12] = xh
    grp[12:24] = xl
    grp[24:36] = xh
    grp[36:39] = ml_dtypes.bfloat16(1.0)
    out = np.zeros((GBASE + XR, GCOLS), ml_dtypes.bfloat16)
    out[0:XR, :] = grp[:, 0:GCOLS]
    out[GBASE : GBASE + XR, :] = grp[:, GCOLS:]
    return out


def _unpack_y(y76):
    """y [76, 384*NSUP] -> [3, npx]. Block h of supertile s lives at
    rows 32*(h%3)+4c+q, cols 384s+128*(h//3)+jj; it holds channel c of
    pixel at j-col J = 1024s + 128h + jj, slot q."""
    v = y76.reshape(76, NSUP, 3, 128)                # (row, s, hc, jj)
    out = np.empty((3, NSLOT, NSUP, 8, 128), np.float32)  # (c,q,s,h,jj)
    for h in range(8):
        pb, ci = 32 * (h % 3), h // 3
        blk = v[pb : pb + 12, :, ci, :]              # (12, s, jj)
        out[:, :, :, h, :] = blk.reshape(3, 4, NSUP, 128)
    o = out.reshape(3, NSLOT, NJ)                    # J = (s, h, jj)
    v2 = o.reshape(3, NSLOT, NT, SLOT_N)             # (c, q, t, n)
    return v2.transpose(0, 2, 1, 3).reshape(3, NPX)


_PROG_CACHE = {}


def kernel(adv_patch, printability_array):
    from concourse.bass_utils import run_bass_kernel_spmd

    adv_patch = np.ascontiguousarray(adv_patch, dtype=np.float32)
    consts = _host_consts(
        np.asarray(printability_array, dtype=np.float32)
    )

    if NSUP not in _PROG_CACHE:
        _PROG_CACHE[NSUP] = _build_program(NSUP)
    nc = _PROG_CACHE[NSUP]

    in_maps = []
    for i in range(NCORES):
        xs = adv_patch[:, i * ROWS : (i + 1) * ROWS, :].reshape(3, NPX)
        in_maps.append({"x": _pack_x(xs), "cb16": consts})

    res = run_bass_kernel_spmd(nc, in_maps, list(range(NCORES)))

    out = np.empty((1, 3, H, W), np.float32)
    for i in range(NCORES):
        out[0, :, i * ROWS : (i + 1) * ROWS, :] = _unpack_y(
            res.results[i]["y"]
        ).reshape(3, ROWS, W)
    return out
